# revision 2
# baseline (speedup 1.0000x reference)
"""CoAttention kernel for Trainium2 (8 NeuronCores, batch-parallel).

Math (per batch b):
    tm = t * mask_t[:, None]; fm = f * mask_f[:, None]
    S  = (tm @ W) @ fm.T                      # [LT, LF] bilinear scores
    C  = tanh(S)  -- only consumed via row/col maxes; tanh is monotonic,
                     so maxes are taken on S and tanh applied to the
                     [512]-vectors afterwards.
    alpha_t = softmax(tanh(rowmax(S)) + (mask_t-1)*BIG)
    alpha_f = softmax(tanh(colmax(S)) + (mask_f-1)*BIG)
    out = alpha_t @ tm + alpha_f @ fm

Implementation choices:
  - batch dim (64) sharded 8-way across cores; 8 batches per core.
  - score chain in bf16 (inputs masked+cast on chip), accumulation fp32.
    The softmax weights only depend on tanh of maxes of S; with these
    input stats |rowmax| >> 3 so results are insensitive to bf16 noise.
  - transposed operands (feature-dim on partitions) produced with the
    DMA xbar transpose (SBUF->SBUF, bf16).
  - softmax without max-subtraction (values bounded by tanh); masked
    entries get bias -80 -> exp ~ 1e-35 ~ 0.
  - final weighted sums: PE matmuls with alpha as the stationary [128,1]
    operand against the masked bf16 naturals, accumulated in fp32 PSUM.
"""

import numpy as np
import ml_dtypes

import concourse.bass as bass
import concourse.tile as tile
from concourse import bacc, mybir
from concourse import masks as cmasks
from concourse.bass_utils import run_bass_kernel_spmd

F32 = mybir.dt.float32
BF16 = mybir.dt.bfloat16
U8 = mybir.dt.uint8
AX = mybir.AxisListType
AF = mybir.ActivationFunctionType

N_CORES = 8
B, LT, LF, D = 64, 512, 512, 512
BL = B // N_CORES          # batches per core
P = 128                    # partitions
NB = D // P                # 128-blocks per 512 dim
BIG = 80.0                 # mask bias (exp(-80) ~ 1e-35; ref uses 1e6, same result)


def _build():
    nc = bacc.Bacc("TRN2", target_bir_lowering=False, debug=False, num_devices=N_CORES)

    t_d = nc.dram_tensor("t", [BL, LT, D], F32, kind="ExternalInput")
    f_d = nc.dram_tensor("f", [BL, LF, D], F32, kind="ExternalInput")
    mt_d = nc.dram_tensor("mask_t", [BL, LT], U8, kind="ExternalInput")
    mf_d = nc.dram_tensor("mask_f", [BL, LF], U8, kind="ExternalInput")
    w_d = nc.dram_tensor("w_beta", [D, D], F32, kind="ExternalInput")
    o_d = nc.dram_tensor("out", [BL, D], F32, kind="ExternalOutput")

    with tile.TileContext(nc) as tc:
        _emit(tc, t_d, f_d, mt_d, mf_d, w_d, o_d)
    nc.compile()
    return nc


def _emit(tc, t_d, f_d, mt_d, mf_d, w_d, o_d):
    nc = tc.nc
    with (
        tc.tile_pool(name="const", bufs=1) as cpool,
        tc.tile_pool(name="nat", bufs=3) as nat_pool,
        tc.tile_pool(name="natbf", bufs=2) as natbf_pool,
        tc.tile_pool(name="tp", bufs=2) as tp_pool,
        tc.tile_pool(name="pjsb", bufs=2) as pjsb_pool,
        tc.tile_pool(name="m1", bufs=2) as m1_pool,
        tc.tile_pool(name="sv", bufs=2) as sv_pool,
        tc.tile_pool(name="pjps", bufs=2, space="PSUM") as pj_ps_pool,
        tc.tile_pool(name="sps", bufs=3, space="PSUM") as s_ps_pool,
        tc.tile_pool(name="mtps", bufs=1, space="PSUM") as m1t_ps_pool,
        tc.tile_pool(name="smps", bufs=2, space="PSUM") as sm_ps_pool,
    ):
        # ---- constants ----
        w_f32 = cpool.tile([P, NB, D], F32)
        # w[d, e] with d = kb*128 + p on (partition, block)
        nc.gpsimd.dma_start(w_f32[:], w_d.ap().rearrange("(kb p) e -> p kb e", p=P))
        w_bf = cpool.tile([P, NB, D], BF16)
        nc.vector.tensor_copy(w_bf[:], w_f32[:])

        ident = cpool.tile([P, P], F32)
        cmasks.make_identity(nc, ident[:])

        ones_col = cpool.tile([P, 1], F32)
        nc.vector.memset(ones_col[:], 1.0)
        ones_row = cpool.tile([1, P], F32)
        nc.vector.memset(ones_row[:], 1.0)

        # masks for all local batches: l = kb*128 + p  ->  [p, b, kb]
        mt_u8 = cpool.tile([P, BL, NB], U8)
        nc.gpsimd.dma_start(mt_u8[:], mt_d.ap().rearrange("b (kb p) -> p b kb", p=P))
        mf_u8 = cpool.tile([P, BL, NB], U8)
        nc.gpsimd.dma_start(mf_u8[:], mf_d.ap().rearrange("b (kb p) -> p b kb", p=P))
        mt_f = cpool.tile([P, BL, NB], F32)
        nc.vector.tensor_copy(mt_f[:], mt_u8[:])
        mf_f = cpool.tile([P, BL, NB], F32)
        nc.vector.tensor_copy(mf_f[:], mf_u8[:])
        bias_t = cpool.tile([P, BL, NB], F32)
        nc.vector.tensor_scalar(
            bias_t[:], mt_f[:], BIG, -BIG,
            op0=mybir.AluOpType.mult, op1=mybir.AluOpType.add,
        )
        bias_f = cpool.tile([P, BL, NB], F32)
        nc.vector.tensor_scalar(
            bias_f[:], mf_f[:], BIG, -BIG,
            op0=mybir.AluOpType.mult, op1=mybir.AluOpType.add,
        )

        for b in range(BL):
            _emit_batch(
                tc, b, t_d, f_d, o_d,
                w_bf, ident, ones_col, ones_row, mt_f, mf_f, bias_t, bias_f,
                nat_pool, natbf_pool, tp_pool, pjsb_pool, m1_pool, sv_pool,
                pj_ps_pool, s_ps_pool, m1t_ps_pool, sm_ps_pool,
            )


def _emit_batch(
    tc, b, t_d, f_d, o_d,
    w_bf, ident, ones_col, ones_row, mt_f, mf_f, bias_t, bias_f,
    nat_pool, natbf_pool, tp_pool, pjsb_pool, m1_pool, sv_pool,
    pj_ps_pool, s_ps_pool, m1t_ps_pool, sm_ps_pool,
):
    nc = tc.nc

    # ---- load naturals: [p, lb, d] with row = lb*128 + p ----
    tnat = nat_pool.tile([P, NB, D], F32, tag="tnat")
    nc.gpsimd.dma_start(tnat[:], t_d.ap()[b].rearrange("(lb p) d -> p lb d", p=P))
    fnat = nat_pool.tile([P, NB, D], F32, tag="fnat")
    nc.gpsimd.dma_start(fnat[:], f_d.ap()[b].rearrange("(lb p) d -> p lb d", p=P))

    # ---- mask + cast to bf16 (DVE for t, ACT for f) ----
    tm_bf = natbf_pool.tile([P, NB, D], BF16, tag="tm_bf")
    fm_bf = natbf_pool.tile([P, NB, D], BF16, tag="fm_bf")
    for lb in range(NB):
        nc.vector.tensor_scalar_mul(
            tm_bf[:, lb, :], tnat[:, lb, :], mt_f[:, b, lb : lb + 1]
        )
        nc.scalar.mul(fm_bf[:, lb, :], fnat[:, lb, :], mf_f[:, b, lb : lb + 1])

    # ---- DMA xbar transposes -> [d_sub, d_blk, l] ----
    tmT = tp_pool.tile([P, NB, LT], BF16, tag="tmT")
    fmT = tp_pool.tile([P, NB, LF], BF16, tag="fmT")
    for lb in range(NB):
        nc.sync.dma_start(
            tmT[:, :, lb * P : (lb + 1) * P], tm_bf[:, lb, :], transpose=True
        )
        nc.sync.dma_start(
            fmT[:, :, lb * P : (lb + 1) * P], fm_bf[:, lb, :], transpose=True
        )

    # ---- matmul 1: projT[e, l] = W.T @ tmT, then evac to bf16 SBUF ----
    projT = pjsb_pool.tile([P, NB, LT], BF16, tag="projT")
    for eb in range(NB):
        pj_ps = pj_ps_pool.tile([P, LT], F32, tag="pj")
        for kb in range(NB):
            nc.tensor.matmul(
                pj_ps[:],
                w_bf[:, kb, eb * P : (eb + 1) * P],
                tmT[:, kb, :],
                start=(kb == 0),
                stop=(kb == NB - 1),
            )
        nc.scalar.copy(projT[:, eb, :], pj_ps[:])

    # ---- matmul 2 + maxes ----
    rm = sv_pool.tile([P, 2 * NB], F32, tag="rm")  # cols 0-3 rowmax, 4-7 colmax
    m1 = m1_pool.tile([P, LF], F32, tag="m1")
    for lb in range(NB):
        s_ps = s_ps_pool.tile([P, LF], F32, tag="s")
        for eb in range(NB):
            nc.tensor.matmul(
                s_ps[:],
                projT[:, eb, lb * P : (lb + 1) * P],
                fmT[:, eb, :],
                start=(eb == 0),
                stop=(eb == NB - 1),
            )
        nc.vector.reduce_max(rm[:, lb : lb + 1], s_ps[:], axis=AX.X)
        if lb == 0:
            nc.vector.tensor_copy(m1[:], s_ps[:])
        else:
            nc.vector.tensor_max(m1[:], s_ps[:], m1[:])

    # colmax: PE-transpose m1 then reduce over l_sub
    m1t_ps = m1t_ps_pool.tile([P, NB, P], F32, tag="m1t")
    for mb in range(NB):
        nc.tensor.transpose(
            m1t_ps[:, mb, :], m1[:, mb * P : (mb + 1) * P], ident[:]
        )
        nc.vector.reduce_max(rm[:, NB + mb : NB + mb + 1], m1t_ps[:, mb, :], axis=AX.X)

    # ---- softmax pieces (no max-subtract; tanh bounds values) ----
    th = sv_pool.tile([P, 2 * NB], F32, tag="th")
    nc.scalar.activation(th[:], rm[:], AF.Tanh)
    tb = sv_pool.tile([P, 2 * NB], F32, tag="tb")
    nc.vector.tensor_add(tb[:, 0:NB], th[:, 0:NB], bias_t[:, b, :])
    nc.vector.tensor_add(tb[:, NB : 2 * NB], th[:, NB : 2 * NB], bias_f[:, b, :])
    ex = sv_pool.tile([P, 2 * NB], F32, tag="ex")
    nc.scalar.activation(ex[:], tb[:], AF.Exp)

    # partition-sum of exps via ones-matmul -> [1, 8]
    sums_ps = sm_ps_pool.tile([1, 2 * NB], F32, tag="sm")
    nc.tensor.matmul(sums_ps[:], ones_col[:], ex[:], start=True, stop=True)
    sums = sv_pool.tile([1, 2], F32, tag="sums")
    nc.vector.reduce_sum(sums[:, 0:1], sums_ps[0:1, 0:NB], axis=AX.X)
    nc.vector.reduce_sum(sums[:, 1:2], sums_ps[0:1, NB : 2 * NB], axis=AX.X)
    rec = sv_pool.tile([1, 2], F32, tag="rec")
    nc.vector.reciprocal(rec[:], sums[:])

    # broadcast recips down partitions via K=1 matmul, then alpha = ex * rec
    rb_ps = sm_ps_pool.tile([P, 2], F32, tag="sm")
    nc.tensor.matmul(rb_ps[:], ones_row[:], rec[:], start=True, stop=True)
    rb = sv_pool.tile([P, 2], F32, tag="rb")
    nc.vector.tensor_copy(rb[:], rb_ps[:])
    alpha = sv_pool.tile([P, 2 * NB], BF16, tag="alpha")
    nc.vector.tensor_scalar_mul(alpha[:, 0:NB], ex[:, 0:NB], rb[:, 0:1])
    nc.vector.tensor_scalar_mul(alpha[:, NB : 2 * NB], ex[:, NB : 2 * NB], rb[:, 1:2])

    # ---- final weighted sums: out = alpha_t @ tm + alpha_f @ fm ----
    out_ps = sm_ps_pool.tile([1, D], F32, tag="sm")
    n_mm = 2 * NB
    k = 0
    for lb in range(NB):
        nc.tensor.matmul(
            out_ps[:], alpha[:, lb : lb + 1], tm_bf[:, lb, :],
            start=(k == 0), stop=(k == n_mm - 1),
        )
        k += 1
    for lb in range(NB):
        nc.tensor.matmul(
            out_ps[:], alpha[:, NB + lb : NB + lb + 1], fm_bf[:, lb, :],
            start=(k == 0), stop=(k == n_mm - 1),
        )
        k += 1

    out_sb = sv_pool.tile([1, D], F32, tag="out_sb")
    nc.scalar.copy(out_sb[:], out_ps[:])
    nc.gpsimd.dma_start(o_d.ap()[b : b + 1, :], out_sb[:])


_NC_CACHE = None


def _get_nc():
    global _NC_CACHE
    if _NC_CACHE is None:
        _NC_CACHE = _build()
    return _NC_CACHE


def kernel(t, f, mask_t, mask_f, w_beta, **_):
    t = np.ascontiguousarray(np.asarray(t), dtype=np.float32)
    f = np.ascontiguousarray(np.asarray(f), dtype=np.float32)
    w = np.ascontiguousarray(np.asarray(w_beta), dtype=np.float32)
    mt = np.ascontiguousarray(np.asarray(mask_t)).astype(np.uint8)
    mf = np.ascontiguousarray(np.asarray(mask_f)).astype(np.uint8)

    nc = _get_nc()
    in_maps = []
    for c in range(N_CORES):
        sl = slice(c * BL, (c + 1) * BL)
        in_maps.append(
            {"t": t[sl], "f": f[sl], "mask_t": mt[sl], "mask_f": mf[sl], "w_beta": w}
        )
    res = run_bass_kernel_spmd(nc, in_maps, core_ids=list(range(N_CORES)))
    return np.concatenate([r["out"] for r in res.results], axis=0)


if __name__ == "__main__":
    rng = np.random.default_rng(0)
    t = rng.standard_normal((B, LT, D), dtype=np.float32)
    f = rng.standard_normal((B, LF, D), dtype=np.float32)
    mask_t = rng.integers(0, 2, (B, LT)).astype(bool)
    mask_f = rng.integers(0, 2, (B, LF)).astype(bool)
    w_beta = (rng.standard_normal((D, D)) * 0.05).astype(np.float32)
    out = kernel(t=t, f=f, mask_t=mask_t, mask_f=mask_f, w_beta=w_beta)
    print("out", out.shape, out.dtype, np.abs(out).mean())
